# revision 59
# baseline (speedup 1.0000x reference)
"""Causal multi-head attention on 8 Trainium2 NeuronCores.

Problem: B=4, H=16, S=2048, D=128, fp32, causal mask.
Sharding: the 64 (batch, head) pairs are split 8-per-core; each core runs
independent attention for its heads. No collectives needed.

Per-core kernel, per head (all matmuls bf16 with fp32 PSUM accumulation):
  - Q^T, K^T staged in SBUF d-major ([d=128, S]), transposed + bf16-cast
    host-side.
  - S^T tiles = K_blk^T.T @ Q^T chunk -> PSUM [k=128, q-cols]   (TensorE)
    Tiles are DP-packed (stream order, bank-bump gaps) into 2-bank PSUM
    groups. Emission is software-pipelined with a 2-group MM1 lookahead and
    1-group PV trail, carried across head boundaries, so exp(G) always
    completes a full period before PV(G) consumes it and the PE never
    starves.
  - exp is split across TWO engines to break the ScalarE ceiling:
      * ScalarE ACTIVATE Exp on most columns (exact), and
      * VectorE via the Schraudolph bit-trick on a tunable fraction of
        off-diagonal columns: u16(s*A + B) IS the bf16 bit pattern of
        exp(s*scale) to ~1.8% rms — one fused mult-add per group.
    Diagonal (masked) tile pairs are exp'd exactly on ScalarE and masked by
    one VectorE multiply with a [tri|1|tri] 0/1 tile.
  - O[qb] += P^T_chunk.T @ [V_blk | 1] -> PSUM [q=128, 129]     (TensorE)
    The ones column accumulates the softmax denominator for free. The two
    128-q-block accumulators of a 256-q chunk share one PSUM bank.
  - O is NOT normalized on device: one copy (VectorE; ScalarE for head-tail
    chunks) evacuates the raw [O | denom] block straight to HBM and the
    host performs the division. This keeps PSUM-bank eviction latency to a
    single hop, which would otherwise stall the next chunk's first matmul.

Causality: only k-blocks at or below the diagonal are computed, and the
moving q-range of diagonal blocks is trimmed to q >= kb*128.
Engine budget per core (measured, profiled): PE ~130us (saturated, 2.4GHz),
ScalarE ~125us, VectorE ~110us, DMA ~93us, +~11us NEFF preamble and ~11us
postamble barrier.
"""

import numpy as np
import ml_dtypes

import concourse.bass as bass
import concourse.mybir as mybir
from concourse import bacc, tile
from concourse.tile import add_dep_helper
from concourse.bass_utils import run_bass_kernel_spmd

B, H, S, D = 4, 16, 2048, 128
N_CORES = 8
HEADS_PER_CORE = (B * H) // N_CORES  # 8
QCHUNK = 256  # q-chunk: 2 query sub-blocks share one 1-bank PSUM O accumulator
NKB = S // 128  # 16 k-blocks per head
VAUG_W = D + 1  # V block columns + ones column
SGRP = 1024  # S^T PSUM group: 2 banks of 512 fp32
O_OFF = (0, 129)  # column offsets of the 2 O accumulators (1 bank)
NJB = QCHUNK // 128  # q sub-blocks per chunk

F32 = mybir.dt.float32
BF16 = mybir.dt.bfloat16
U16 = mybir.dt.uint16

# Fraction of each S^T group's exp computed on DVE via the Schraudolph
# bit-trick (bf16_bits = u16(s*A + B)); the rest runs on ScalarE. Splitting
# moves the exp bottleneck off the Activation engine. Error: rms 1.8% on the
# DVE share; measured end-to-end ~1e-2 at 0.40 vs the 2e-2 gate.
EXP_DVE_FRAC = 0.46  # fraction of PLAIN (non-diagonal) columns exp'd on DVE
SCHRAUDOLPH_A = float(np.log2(np.e) / np.sqrt(np.float32(128)) * 128.0)
SCHRAUDOLPH_B = 16248.75
OW = NJB * VAUG_W  # 258: unnormalized O block + denominator columns

_COMPILED = {}


def _arrange(units):
    """Sequential stream-order placement of (key, w) units; a unit that
    would straddle a 512-col PSUM bank boundary is bumped to the next bank
    (the gap columns are exp'd harmlessly and never read downstream).

    Returns [(key, w, pos)] or None if the span exceeds SGRP.
    """
    placed = []
    off = 0
    for k, w in units:
        if off // 512 != (off + w - 1) // 512:
            off = (off // 512 + 1) * 512
        placed.append((k, w, off))
        off += w
    if off > SGRP:
        return None
    return placed


def _head_stream():
    """All of one head's S^T tiles in emission order: [(qc, kb, width)]."""
    stream = []
    for qc in range(S // QCHUNK):
        q_base = qc * QCHUNK
        for kb in range(q_base // 128 + QCHUNK // 128):
            q_lo = max(q_base, kb * 128)
            stream.append((qc, kb, q_base + QCHUNK - q_lo))
    return stream


def _pack_stream():
    """DP-optimal partition of the head's tile stream into single-run PSUM
    groups (may span one chunk boundary; o_ps is double-buffered). The two
    diagonal (masked) tiles of each chunk are fused into one adjacent unit
    so a single masked-exp op covers both.

    Returns a list of groups: [(n_cols, [(qc, kb, width, pos)])].
    """
    stream = _head_stream()
    n = len(stream)

    def diag_first(idx):
        qc, kb, w = stream[idx]
        return kb * 128 == qc * QCHUNK

    feas = {}
    for i in range(n):
        for j in range(i + 1, n + 1):
            win = stream[i:j]
            if sum(w for _, _, w in win) > SGRP:
                break
            if len({qc for qc, _, _ in win}) > 2:
                break
            if diag_first(j - 1):
                continue  # would split a diagonal pair across groups
            units = []
            k = i
            while k < j:
                if diag_first(k):
                    units.append((k, stream[k][2] + stream[k + 1][2]))
                    k += 2
                else:
                    units.append((k, stream[k][2]))
                    k += 1
            placed = _arrange(units)
            if placed is not None:
                feas[(i, j)] = placed

    INF = 1 << 30
    best = [INF] * (n + 1)
    best[0] = 0
    prev = [None] * (n + 1)
    for j in range(1, n + 1):
        for i in range(j):
            if (i, j) in feas and best[i] + 1 < best[j]:
                best[j] = best[i] + 1
                prev[j] = i
    groups = []
    j = n
    while j > 0:
        i = prev[j]
        placed = feas[(i, j)]
        items = []
        for k, w, pos in placed:
            qc, kb, tw = stream[k]
            items.append((qc, kb, tw, pos))
            if w != tw:  # fused diagonal pair: partner sits right after
                qc2, kb2, tw2 = stream[k + 1]
                items.append((qc2, kb2, tw2, pos + tw))
        n_cols = max(pos + w for _, w, pos in placed)
        groups.append((n_cols, items))
        j = i
    groups.reverse()
    return groups


def _build_program(repeat=1):
    """Build + compile the per-core Bass program. Returns the Bacc module."""
    nc = bacc.Bacc(None)

    qT = nc.declare_dram_parameter(
        "qT", [HEADS_PER_CORE, 128, S], BF16, isOutput=False
    )
    kT = nc.declare_dram_parameter(
        "kT", [HEADS_PER_CORE, 128, S], BF16, isOutput=False
    )
    vaug = nc.declare_dram_parameter(
        "vaug", [HEADS_PER_CORE, 128, NKB, VAUG_W], BF16, isOutput=False
    )
    # Permuted UNNORMALIZED output (incl. denominator columns); host divides:
    # out[h, qc, p, j*VAUG_W + d] = O_raw[h, qc*QCHUNK + j*128 + p, d]
    # out[h, qc, p, j*VAUG_W + D] = sum_k p[k, q]  (softmax denominator)
    out = nc.declare_dram_parameter(
        "out", [HEADS_PER_CORE, S // QCHUNK, 128, OW], F32, isOutput=True
    )

    # Keep-mask for a fused diagonal pair [256-wide tile | 128-wide tile]
    # in S^T coords: keep[k, q] = 1.0 if k <= q else 0.0 (middle 128 cols of
    # the 256-wide tile are fully below the diagonal -> all ones).
    keep = np.tril(np.ones((128, 128), dtype=np.float32)).T
    tri = np.ones((128, 384), dtype=np.float32)
    tri[:, 0:128] = keep
    tri[:, 256:384] = keep
    tri_dram = nc.inline_tensor(
        np.ascontiguousarray(tri.astype(ml_dtypes.bfloat16)), name="tri01"
    )

    scale = float(1.0 / np.sqrt(np.float32(D)))

    with tile.TileContext(nc) as tc:
        with (
            tc.tile_pool(name="consts", bufs=1) as consts,
            tc.tile_pool(name="heads", bufs=3) as heads,
            tc.tile_pool(name="p", bufs=8) as ppool,
            tc.tile_pool(name="o", bufs=8) as opool,
            tc.tile_pool(name="spsum", bufs=3, space="PSUM") as spsum,
            tc.tile_pool(name="opsum", bufs=2, space="PSUM") as opsum,
        ):
            tri_sb = consts.tile([128, 384], BF16)

            def load_head(h):
                qT_sb = heads.tile([128, S], BF16, tag="qT", name="qT_sb")
                kT_sb = heads.tile([128, S], BF16, tag="kT", name="kT_sb")
                vaug_sb = heads.tile(
                    [128, NKB * VAUG_W], BF16, tag="vaug", name="vaug_sb"
                )
                # split loads so the first S^T groups' inputs land early
                # (subtile deps let matmuls start before the tail arrives);
                # chunk 0 only needs the first 256 cols of kT/qT, and the
                # first PVs only the first few V blocks
                nc.sync.dma_start(kT_sb[:, :128], kT[h][:, :128])
                nc.sync.dma_start(qT_sb[:, :256], qT[h][:, :256])
                nc.sync.dma_start(kT_sb[:, 128:256], kT[h][:, 128:256])
                # vaug rides ahead of the qT tail: the first PV of each head
                # measures ~0.5us blocked on exactly this slab's arrival
                nc.sync.dma_start(
                    vaug_sb[:, : 4 * VAUG_W],
                    vaug[h][:, :4].rearrange("p n m -> p (n m)"),
                )
                nc.sync.dma_start(qT_sb[:, 256:512], qT[h][:, 256:512])
                if h == 0:
                    # tiny; must beat the first diagonal mask (group 0's PV)
                    nc.sync.dma_start(tri_sb[:], tri_dram[:])
                nc.sync.dma_start(kT_sb[:, 256:1024], kT[h][:, 256:1024])
                nc.sync.dma_start(qT_sb[:, 512:1024], qT[h][:, 512:1024])
                nc.sync.dma_start(kT_sb[:, 1024:], kT[h][:, 1024:])
                nc.sync.dma_start(qT_sb[:, 1024:], qT[h][:, 1024:])
                nc.sync.dma_start(
                    vaug_sb[:, 4 * VAUG_W :],
                    vaug[h][:, 4:].rearrange("p n m -> p (n m)"),
                )
                return qT_sb, kT_sb, vaug_sb

            def body():
                groups = _pack_stream()  # identical for every head
                n_g = len(groups)
                total = HEADS_PER_CORE * n_g



                # Per-head emission context, created lazily on first touch
                # (which happens via the mm1 lookahead one group early).
                # Creating ctx(h) also prefetches head h+1's DMA loads.
                loads = {0: load_head(0)}
                # running deficit of plain columns owed to the DVE exp path
                dve_owed = [0.0]
                ctxs = {}

                def get_ctx(h):
                    if h in ctxs:
                        return ctxs[h]
                    qT_sb, kT_sb, vaug_sb = loads.pop(h)
                    if h + 1 < HEADS_PER_CORE and h + 1 not in loads:
                        loads[h + 1] = load_head(h + 1)
                    o_chunks = {}  # qc -> [o_ps tile, prev_mm2 chain tail]

                    def emit_mm1(g_idx):
                        # S^T matmuls for one group; returns its s_ps tile.
                        # start=True lazily zeroes a whole 2KB PSUM bank, so
                        # only the first tile landing in each bank may start,
                        # and only the last may stop; same-bank order pinned.
                        n_cols, g_items = groups[g_idx]
                        s_ps = spsum.tile(
                            [128, SGRP], F32, tag="s_grp", name="s_ps"
                        )
                        bank_last = {}
                        for idx, (qc, kb, w, pos) in enumerate(g_items):
                            bank_last[pos // 512] = idx
                        seen_banks = set()
                        prev_mm1 = None
                        for idx, (qc, kb, w, pos) in enumerate(g_items):
                            b = pos // 512
                            first = b not in seen_banks
                            seen_banks.add(b)
                            q_lo = max(qc * QCHUNK, kb * 128)
                            mm = nc.tensor.matmul(
                                s_ps[:, pos : pos + w],
                                kT_sb[:, kb * 128 : (kb + 1) * 128],
                                qT_sb[:, q_lo : q_lo + w],
                                start=first,
                                stop=(idx == bank_last[b]),
                            )
                            if prev_mm1 is not None:
                                add_dep_helper(
                                    mm.ins, prev_mm1, reason="zero-region order"
                                )
                            prev_mm1 = mm.ins
                        return s_ps

                    def emit_pv(p_sb, g_items):
                        # Both O accumulators (j0, j1) live in one PSUM bank.
                        # One start (zeroing the bank) on its first-touched
                        # matmul; one stop on its last (j1's diagonal tail).
                        for qc, kb, w, pos in g_items:
                            if qc not in o_chunks:
                                o_chunks[qc] = [
                                    opsum.tile(
                                        [128, 2 * 129], F32,
                                        tag="o_ps", name="o_ps",
                                    ),
                                    None,
                                ]
                            o_ent = o_chunks[qc]
                            q_base = qc * QCHUNK
                            q_lo = max(q_base, kb * 128)
                            j_lo = (q_lo - q_base) // 128
                            for j in range(j_lo, NJB):
                                off = pos + j * 128 - (q_lo - q_base)
                                qb_g = q_base // 128 + j
                                st = kb == 0 and j == 0
                                sp = j == NJB - 1 and kb == qb_g
                                mm = nc.tensor.matmul(
                                    o_ent[0][:, O_OFF[j] : O_OFF[j] + VAUG_W],
                                    p_sb[:, off : off + 128],
                                    vaug_sb[:, kb * VAUG_W : (kb + 1) * VAUG_W],
                                    start=st,
                                    stop=sp,
                                )
                                if o_ent[1] is not None:
                                    add_dep_helper(
                                        mm.ins, o_ent[1], reason="zero-region order"
                                    )
                                o_ent[1] = mm.ins
                        # evacuate + store any chunk whose diagonal tail
                        # (kb == last qb, width 128) was consumed by this
                        # group; normalization happens host-side, so a single
                        # copy frees the PSUM bank with minimal latency
                        for qc, kb, w, pos in g_items:
                            if kb != qc * NJB + NJB - 1:
                                continue
                            o_ps = o_chunks.pop(qc)[0]
                            o_sb = opool.tile(
                                [128, OW], F32, tag="o_sb", name="o_sb"
                            )
                            # the head-tail chunks evacuate on ScalarE (idle
                            # around head boundaries; their banks gate the
                            # next head's first PVs); the rest on VectorE
                            if qc >= S // QCHUNK - 2:
                                nc.scalar.copy(o_sb[:], o_ps[:])
                            else:
                                nc.vector.tensor_copy(o_sb[:], o_ps[:])
                            nc.sync.dma_start(out[h, qc], o_sb[:])

                    ctxs[h] = (emit_mm1, emit_pv)
                    return ctxs[h]

                # Flattened (head, group) task loop: the software pipeline
                # (mm1 lookahead +2, PV trailing -1) is carried ACROSS head
                # boundaries so ACT never sees a refill bubble, and the exp
                # of group G completes a full period before PV(G) needs it.
                s_q = {}
                pend = None  # (emit_pv, p_sb, g_items)
                for t in range(total):
                    for tt in ((t, t + 1, t + 2) if t == 0 else (t + 2,)):
                        if tt < total and tt not in s_q:
                            h2, g2 = divmod(tt, n_g)
                            s_q[tt] = get_ctx(h2)[0](g2)
                    h, g_idx = divmod(t, n_g)
                    emit_mm1, emit_pv = get_ctx(h)
                    n_cols, g_items = groups[g_idx]
                    s_ps = s_q.pop(t)
                    p_sb = ppool.tile(
                        [128, SGRP], BF16, tag="p_sb", name="p_sb"
                    )
                    # The group span is [0, n_cols): a leading run of plain
                    # columns goes to DVE (Schraudolph exp), the rest — pairs
                    # included — to one ScalarE Exp. Diagonal pairs are then
                    # masked in place by one DVE multiply each. Gap columns
                    # are exp'd garbage that nothing reads.
                    pairs = [
                        (pos, w + 128)
                        for qc, kb, w, pos in g_items
                        if kb * 128 == qc * QCHUNK
                    ]
                    pairs.sort()
                    plain_cols = n_cols - 384 * len(pairs)
                    dve_owed[0] += plain_cols * EXP_DVE_FRAC
                    first_pair = pairs[0][0] if pairs else n_cols
                    # the first groups of each head run all-ScalarE: around
                    # head boundaries the pipeline lead collapses and the
                    # DVE leg's latency would stall PV; the owed columns
                    # shift to mid-head groups where there is slack
                    if g_idx < 2:
                        first_pair = 0
                    c = min(int(dve_owed[0]), first_pair)
                    dve_owed[0] -= c
                    if c > 0:
                        nc.vector.tensor_scalar(
                            p_sb[:, 0:c].bitcast(U16),
                            s_ps[:, 0:c],
                            SCHRAUDOLPH_A,
                            SCHRAUDOLPH_B,
                            mybir.AluOpType.mult,
                            mybir.AluOpType.add,
                        )
                    if c < n_cols:
                        nc.scalar.activation(
                            p_sb[:, c:n_cols],
                            s_ps[:, c:n_cols],
                            mybir.ActivationFunctionType.Exp,
                            scale=scale,
                        )
                    for pos, w in pairs:
                        nc.vector.tensor_mul(
                            p_sb[:, pos : pos + w],
                            p_sb[:, pos : pos + w],
                            tri_sb[:, :w],
                        )
                    if pend is not None:
                        pend[0](pend[1], pend[2])
                    pend = (emit_pv, p_sb, g_items)
                # flush the last group's PV + norm
                pend[0](pend[1], pend[2])
                ctxs.clear()
                loads.clear()

            if repeat > 1:
                with tc.For_i(
                    0,
                    repeat,
                    1,
                    hint_engines=(
                        mybir.EngineType.PE,
                        mybir.EngineType.Activation,
                        mybir.EngineType.DVE,
                        mybir.EngineType.SP,
                        mybir.EngineType.Pool,
                    ),
                ):
                    body()
            else:
                body()

    nc.compile()
    return nc


def _causal_mask_ok(mask: np.ndarray) -> bool:
    m = np.asarray(mask).reshape(S, S)
    expect = np.triu(np.ones((S, S), dtype=bool), k=1)
    return bool((m == expect).all())


def _numpy_fallback(keys, queries, values, mask):
    """Host reference for non-causal masks (robustness insurance)."""
    out = np.empty((B, H, S, D), dtype=np.float32)
    scale = 1.0 / np.sqrt(np.float32(D))
    m = np.asarray(mask).reshape(S, S)
    for b in range(B):
        for h in range(H):
            logits = (queries[b, h] @ keys[b, h].T) * scale
            logits = np.where(m, -np.inf, logits)
            logits -= logits.max(axis=-1, keepdims=True)
            p = np.exp(logits)
            p /= p.sum(axis=-1, keepdims=True)
            out[b, h] = p @ values[b, h]
    return out


def prepare_in_maps(keys, queries, values):
    keys = np.ascontiguousarray(np.asarray(keys, dtype=np.float32))
    queries = np.ascontiguousarray(np.asarray(queries, dtype=np.float32))
    values = np.ascontiguousarray(np.asarray(values, dtype=np.float32))

    # [B,H,...] -> [64, ...] head-pair-major, then 8 heads per core
    q_flat = queries.reshape(B * H, S, D)
    k_flat = keys.reshape(B * H, S, D)
    v_flat = values.reshape(B * H, S, D)

    in_maps = []
    for c in range(N_CORES):
        sl = slice(c * HEADS_PER_CORE, (c + 1) * HEADS_PER_CORE)
        in_maps.append(make_core_inputs(q_flat[sl], k_flat[sl], v_flat[sl]))
    return in_maps


def make_core_inputs(q, k, v):
    """Per-core in_map from [heads, S, D] fp32 arrays."""
    bf = ml_dtypes.bfloat16
    qT = np.ascontiguousarray(q.transpose(0, 2, 1)).astype(bf)
    kT = np.ascontiguousarray(k.transpose(0, 2, 1)).astype(bf)
    vaug = np.zeros((HEADS_PER_CORE, 128, NKB, VAUG_W), dtype=bf)
    # vaug[h, k_local, kb, :128] = V[h, kb*128 + k_local, :]
    vaug[:, :, :, :D] = (
        v.reshape(HEADS_PER_CORE, NKB, 128, D).transpose(0, 2, 1, 3).astype(bf)
    )
    vaug[:, :, :, D] = 1.0
    return {"qT": qT, "kT": kT, "vaug": vaug}


def kernel(keys, queries, values, mask):
    if not _causal_mask_ok(mask):
        return _numpy_fallback(
            np.asarray(keys, dtype=np.float32),
            np.asarray(queries, dtype=np.float32),
            np.asarray(values, dtype=np.float32),
            mask,
        )

    if "nc" not in _COMPILED:
        _COMPILED["nc"] = _build_program()
    nc = _COMPILED["nc"]

    in_maps = prepare_in_maps(keys, queries, values)

    res = None
    last_err = None
    for _attempt in range(3):
        try:
            res = run_bass_kernel_spmd(
                nc, in_maps, core_ids=list(range(N_CORES))
            )
            break
        except Exception as e:  # flaky device state: retry
            last_err = e
    if res is None:
        raise last_err

    out = np.concatenate(
        [res.results[c]["out"][None] for c in range(N_CORES)], axis=0
    )  # [n_cores, heads, S//QCHUNK, 128, OW] permuted, unnormalized
    out = out.reshape(
        N_CORES, HEADS_PER_CORE, S // QCHUNK, 128, NJB, VAUG_W
    )
    o = out[..., :D] / out[..., D : D + 1]  # softmax normalization
    o = o.transpose(0, 1, 2, 4, 3, 5).reshape(B, H, S, D)
    return np.ascontiguousarray(o)



# revision 61
# speedup vs baseline: 1.0035x; 1.0035x over previous
"""Causal multi-head attention on 8 Trainium2 NeuronCores.

Problem: B=4, H=16, S=2048, D=128, fp32, causal mask.
Sharding: the 64 (batch, head) pairs are split 8-per-core; each core runs
independent attention for its heads. No collectives needed.

Per-core kernel, per head (all matmuls bf16 with fp32 PSUM accumulation):
  - Q^T, K^T staged in SBUF d-major ([d=128, S]), transposed + bf16-cast
    host-side.
  - S^T tiles = K_blk^T.T @ Q^T chunk -> PSUM [k=128, q-cols]   (TensorE)
    Tiles are DP-packed (stream order, bank-bump gaps) into 2-bank PSUM
    groups. Emission is software-pipelined with a 2-group MM1 lookahead and
    1-group PV trail, carried across head boundaries, so exp(G) always
    completes a full period before PV(G) consumes it and the PE never
    starves.
  - exp is split across TWO engines to break the ScalarE ceiling:
      * ScalarE ACTIVATE Exp on most columns (exact), and
      * VectorE via the Schraudolph bit-trick on a tunable fraction of
        off-diagonal columns: u16(s*A + B) IS the bf16 bit pattern of
        exp(s*scale) to ~1.8% rms — one fused mult-add per group.
    Diagonal (masked) tile pairs are exp'd exactly on ScalarE and masked by
    one VectorE multiply with a [tri|1|tri] 0/1 tile.
  - O[qb] += P^T_chunk.T @ [V_blk | 1] -> PSUM [q=128, 129]     (TensorE)
    The ones column accumulates the softmax denominator for free. The two
    128-q-block accumulators of a 256-q chunk share one PSUM bank.
  - O is NOT normalized on device: one copy (VectorE; ScalarE for head-tail
    chunks) evacuates the raw [O | denom] block straight to HBM and the
    host performs the division. This keeps PSUM-bank eviction latency to a
    single hop, which would otherwise stall the next chunk's first matmul.

Causality: only k-blocks at or below the diagonal are computed, and the
moving q-range of diagonal blocks is trimmed to q >= kb*128.
Engine budget per core (measured, profiled): PE ~130us (saturated, 2.4GHz),
ScalarE ~125us, VectorE ~110us, DMA ~93us, +~11us NEFF preamble and ~11us
postamble barrier.
"""

import numpy as np
import ml_dtypes

import concourse.bass as bass
import concourse.mybir as mybir
from concourse import bacc, tile
from concourse.tile import add_dep_helper
from concourse.bass_utils import run_bass_kernel_spmd

B, H, S, D = 4, 16, 2048, 128
N_CORES = 8
HEADS_PER_CORE = (B * H) // N_CORES  # 8
QCHUNK = 256  # q-chunk: 2 query sub-blocks share one 1-bank PSUM O accumulator
NKB = S // 128  # 16 k-blocks per head
VAUG_W = D + 1  # V block columns + ones column
SGRP = 1024  # S^T PSUM group: 2 banks of 512 fp32
O_OFF = (0, 129)  # column offsets of the 2 O accumulators (1 bank)
NJB = QCHUNK // 128  # q sub-blocks per chunk

F32 = mybir.dt.float32
BF16 = mybir.dt.bfloat16
U16 = mybir.dt.uint16

# Fraction of each S^T group's exp computed on DVE via the Schraudolph
# bit-trick (bf16_bits = u16(s*A + B)); the rest runs on ScalarE. Splitting
# moves the exp bottleneck off the Activation engine. Error: rms 1.8% on the
# DVE share; measured end-to-end ~1e-2 at 0.40 vs the 2e-2 gate.
EXP_DVE_FRAC = 0.46  # fraction of PLAIN (non-diagonal) columns exp'd on DVE
SCHRAUDOLPH_A = float(np.log2(np.e) / np.sqrt(np.float32(128)) * 128.0)
SCHRAUDOLPH_B = 16248.75
OW = NJB * VAUG_W  # 258: unnormalized O block + denominator columns

_COMPILED = {}


def _arrange(units):
    """Sequential stream-order placement of (key, w) units; a unit that
    would straddle a 512-col PSUM bank boundary is bumped to the next bank
    (the gap columns are exp'd harmlessly and never read downstream).

    Returns [(key, w, pos)] or None if the span exceeds SGRP.
    """
    placed = []
    off = 0
    for k, w in units:
        if off // 512 != (off + w - 1) // 512:
            off = (off // 512 + 1) * 512
        placed.append((k, w, off))
        off += w
    if off > SGRP:
        return None
    return placed


def _head_stream():
    """All of one head's S^T tiles in emission order: [(qc, kb, width)]."""
    stream = []
    for qc in range(S // QCHUNK):
        q_base = qc * QCHUNK
        for kb in range(q_base // 128 + QCHUNK // 128):
            q_lo = max(q_base, kb * 128)
            stream.append((qc, kb, q_base + QCHUNK - q_lo))
    return stream


def _pack_stream():
    """DP-optimal partition of the head's tile stream into single-run PSUM
    groups (may span one chunk boundary; o_ps is double-buffered). The two
    diagonal (masked) tiles of each chunk are fused into one adjacent unit
    so a single masked-exp op covers both.

    Returns a list of groups: [(n_cols, [(qc, kb, width, pos)])].
    """
    stream = _head_stream()
    n = len(stream)

    def diag_first(idx):
        qc, kb, w = stream[idx]
        return kb * 128 == qc * QCHUNK

    feas = {}
    for i in range(n):
        for j in range(i + 1, n + 1):
            win = stream[i:j]
            if sum(w for _, _, w in win) > SGRP:
                break
            if len({qc for qc, _, _ in win}) > 2:
                break
            if diag_first(j - 1):
                continue  # would split a diagonal pair across groups
            units = []
            k = i
            while k < j:
                if diag_first(k):
                    units.append((k, stream[k][2] + stream[k + 1][2]))
                    k += 2
                else:
                    units.append((k, stream[k][2]))
                    k += 1
            placed = _arrange(units)
            if placed is not None:
                feas[(i, j)] = placed

    INF = 1 << 30
    best = [INF] * (n + 1)
    best[0] = 0
    prev = [None] * (n + 1)
    for j in range(1, n + 1):
        for i in range(j):
            if (i, j) in feas and best[i] + 1 < best[j]:
                best[j] = best[i] + 1
                prev[j] = i
    groups = []
    j = n
    while j > 0:
        i = prev[j]
        placed = feas[(i, j)]
        items = []
        for k, w, pos in placed:
            qc, kb, tw = stream[k]
            items.append((qc, kb, tw, pos))
            if w != tw:  # fused diagonal pair: partner sits right after
                qc2, kb2, tw2 = stream[k + 1]
                items.append((qc2, kb2, tw2, pos + tw))
        n_cols = max(pos + w for _, w, pos in placed)
        groups.append((n_cols, items))
        j = i
    groups.reverse()
    return groups


def _build_program(repeat=1):
    """Build + compile the per-core Bass program. Returns the Bacc module."""
    nc = bacc.Bacc(None)

    qT = nc.declare_dram_parameter(
        "qT", [HEADS_PER_CORE, 128, S], BF16, isOutput=False
    )
    kT = nc.declare_dram_parameter(
        "kT", [HEADS_PER_CORE, 128, S], BF16, isOutput=False
    )
    vaug = nc.declare_dram_parameter(
        "vaug", [HEADS_PER_CORE, 128, NKB, VAUG_W], BF16, isOutput=False
    )
    # Permuted UNNORMALIZED output (incl. denominator columns); host divides:
    # out[h, qc, p, j*VAUG_W + d] = O_raw[h, qc*QCHUNK + j*128 + p, d]
    # out[h, qc, p, j*VAUG_W + D] = sum_k p[k, q]  (softmax denominator)
    out = nc.declare_dram_parameter(
        "out", [HEADS_PER_CORE, S // QCHUNK, 128, OW], F32, isOutput=True
    )

    # Keep-mask for a fused diagonal pair [256-wide tile | 128-wide tile]
    # in S^T coords: keep[k, q] = 1.0 if k <= q else 0.0 (middle 128 cols of
    # the 256-wide tile are fully below the diagonal -> all ones).
    keep = np.tril(np.ones((128, 128), dtype=np.float32)).T
    tri = np.ones((128, 384), dtype=np.float32)
    tri[:, 0:128] = keep
    tri[:, 256:384] = keep
    tri_dram = nc.inline_tensor(
        np.ascontiguousarray(tri.astype(ml_dtypes.bfloat16)), name="tri01"
    )

    scale = float(1.0 / np.sqrt(np.float32(D)))

    with tile.TileContext(nc) as tc:
        with (
            tc.tile_pool(name="consts", bufs=1) as consts,
            tc.tile_pool(name="heads", bufs=4) as heads,
            tc.tile_pool(name="p", bufs=8) as ppool,
            tc.tile_pool(name="o", bufs=8) as opool,
            tc.tile_pool(name="spsum", bufs=3, space="PSUM") as spsum,
            tc.tile_pool(name="opsum", bufs=2, space="PSUM") as opsum,
        ):
            tri_sb = consts.tile([128, 384], BF16)

            def load_head(h):
                qT_sb = heads.tile([128, S], BF16, tag="qT", name="qT_sb")
                kT_sb = heads.tile([128, S], BF16, tag="kT", name="kT_sb")
                vaug_sb = heads.tile(
                    [128, NKB * VAUG_W], BF16, tag="vaug", name="vaug_sb"
                )
                # split loads so the first S^T groups' inputs land early
                # (subtile deps let matmuls start before the tail arrives);
                # chunk 0 only needs the first 256 cols of kT/qT, and the
                # first PVs only the first few V blocks
                nc.sync.dma_start(kT_sb[:, :128], kT[h][:, :128])
                nc.sync.dma_start(qT_sb[:, :256], qT[h][:, :256])
                nc.sync.dma_start(kT_sb[:, 128:256], kT[h][:, 128:256])
                # vaug rides ahead of the qT tail: the first PV of each head
                # measures ~0.5us blocked on exactly this slab's arrival
                nc.sync.dma_start(
                    vaug_sb[:, : 4 * VAUG_W],
                    vaug[h][:, :4].rearrange("p n m -> p (n m)"),
                )
                nc.sync.dma_start(qT_sb[:, 256:512], qT[h][:, 256:512])
                if h == 0:
                    # tiny; must beat the first diagonal mask (group 0's PV)
                    nc.sync.dma_start(tri_sb[:], tri_dram[:])
                nc.sync.dma_start(kT_sb[:, 256:1024], kT[h][:, 256:1024])
                nc.sync.dma_start(qT_sb[:, 512:1024], qT[h][:, 512:1024])
                nc.sync.dma_start(kT_sb[:, 1024:], kT[h][:, 1024:])
                nc.sync.dma_start(qT_sb[:, 1024:], qT[h][:, 1024:])
                nc.sync.dma_start(
                    vaug_sb[:, 4 * VAUG_W :],
                    vaug[h][:, 4:].rearrange("p n m -> p (n m)"),
                )
                return qT_sb, kT_sb, vaug_sb

            def body():
                groups = _pack_stream()  # identical for every head
                n_g = len(groups)
                total = HEADS_PER_CORE * n_g



                # Per-head emission context, created lazily on first touch
                # (which happens via the mm1 lookahead one group early).
                # Creating ctx(h) also prefetches head h+1's DMA loads.
                loads = {0: load_head(0)}
                # running deficit of plain columns owed to the DVE exp path
                dve_owed = [0.0]
                ctxs = {}

                def get_ctx(h):
                    if h in ctxs:
                        return ctxs[h]
                    qT_sb, kT_sb, vaug_sb = loads.pop(h)
                    for hn in (h + 1, h + 2):
                        if hn < HEADS_PER_CORE and hn not in loads:
                            loads[hn] = load_head(hn)
                    o_chunks = {}  # qc -> [o_ps tile, prev_mm2 chain tail]

                    def emit_mm1(g_idx):
                        # S^T matmuls for one group; returns its s_ps tile.
                        # start=True lazily zeroes a whole 2KB PSUM bank, so
                        # only the first tile landing in each bank may start,
                        # and only the last may stop; same-bank order pinned.
                        n_cols, g_items = groups[g_idx]
                        s_ps = spsum.tile(
                            [128, SGRP], F32, tag="s_grp", name="s_ps"
                        )
                        bank_last = {}
                        for idx, (qc, kb, w, pos) in enumerate(g_items):
                            bank_last[pos // 512] = idx
                        seen_banks = set()
                        prev_mm1 = None
                        for idx, (qc, kb, w, pos) in enumerate(g_items):
                            b = pos // 512
                            first = b not in seen_banks
                            seen_banks.add(b)
                            q_lo = max(qc * QCHUNK, kb * 128)
                            mm = nc.tensor.matmul(
                                s_ps[:, pos : pos + w],
                                kT_sb[:, kb * 128 : (kb + 1) * 128],
                                qT_sb[:, q_lo : q_lo + w],
                                start=first,
                                stop=(idx == bank_last[b]),
                            )
                            if prev_mm1 is not None:
                                add_dep_helper(
                                    mm.ins, prev_mm1, reason="zero-region order"
                                )
                            prev_mm1 = mm.ins
                        return s_ps

                    def emit_pv(p_sb, g_items):
                        # Both O accumulators (j0, j1) live in one PSUM bank.
                        # One start (zeroing the bank) on its first-touched
                        # matmul; one stop on its last (j1's diagonal tail).
                        for qc, kb, w, pos in g_items:
                            if qc not in o_chunks:
                                o_chunks[qc] = [
                                    opsum.tile(
                                        [128, 2 * 129], F32,
                                        tag="o_ps", name="o_ps",
                                    ),
                                    None,
                                ]
                            o_ent = o_chunks[qc]
                            q_base = qc * QCHUNK
                            q_lo = max(q_base, kb * 128)
                            j_lo = (q_lo - q_base) // 128
                            for j in range(j_lo, NJB):
                                off = pos + j * 128 - (q_lo - q_base)
                                qb_g = q_base // 128 + j
                                st = kb == 0 and j == 0
                                sp = j == NJB - 1 and kb == qb_g
                                mm = nc.tensor.matmul(
                                    o_ent[0][:, O_OFF[j] : O_OFF[j] + VAUG_W],
                                    p_sb[:, off : off + 128],
                                    vaug_sb[:, kb * VAUG_W : (kb + 1) * VAUG_W],
                                    start=st,
                                    stop=sp,
                                )
                                if o_ent[1] is not None:
                                    add_dep_helper(
                                        mm.ins, o_ent[1], reason="zero-region order"
                                    )
                                o_ent[1] = mm.ins
                        # evacuate + store any chunk whose diagonal tail
                        # (kb == last qb, width 128) was consumed by this
                        # group; normalization happens host-side, so a single
                        # copy frees the PSUM bank with minimal latency
                        for qc, kb, w, pos in g_items:
                            if kb != qc * NJB + NJB - 1:
                                continue
                            o_ps = o_chunks.pop(qc)[0]
                            o_sb = opool.tile(
                                [128, OW], F32, tag="o_sb", name="o_sb"
                            )
                            # the head-tail chunks evacuate on ScalarE (idle
                            # around head boundaries; their banks gate the
                            # next head's first PVs); the rest on VectorE
                            if qc >= S // QCHUNK - 2:
                                nc.scalar.copy(o_sb[:], o_ps[:])
                            else:
                                nc.vector.tensor_copy(o_sb[:], o_ps[:])
                            nc.sync.dma_start(out[h, qc], o_sb[:])

                    ctxs[h] = (emit_mm1, emit_pv)
                    return ctxs[h]

                # Flattened (head, group) task loop: the software pipeline
                # (mm1 lookahead +2, PV trailing -1) is carried ACROSS head
                # boundaries so ACT never sees a refill bubble, and the exp
                # of group G completes a full period before PV(G) needs it.
                s_q = {}
                pend = None  # (emit_pv, p_sb, g_items)
                for t in range(total):
                    for tt in ((t, t + 1, t + 2) if t == 0 else (t + 2,)):
                        if tt < total and tt not in s_q:
                            h2, g2 = divmod(tt, n_g)
                            s_q[tt] = get_ctx(h2)[0](g2)
                    h, g_idx = divmod(t, n_g)
                    emit_mm1, emit_pv = get_ctx(h)
                    n_cols, g_items = groups[g_idx]
                    s_ps = s_q.pop(t)
                    p_sb = ppool.tile(
                        [128, SGRP], BF16, tag="p_sb", name="p_sb"
                    )
                    # The group span is [0, n_cols): a leading run of plain
                    # columns goes to DVE (Schraudolph exp), the rest — pairs
                    # included — to one ScalarE Exp. Diagonal pairs are then
                    # masked in place by one DVE multiply each. Gap columns
                    # are exp'd garbage that nothing reads.
                    pairs = [
                        (pos, w + 128)
                        for qc, kb, w, pos in g_items
                        if kb * 128 == qc * QCHUNK
                    ]
                    pairs.sort()
                    plain_cols = n_cols - 384 * len(pairs)
                    dve_owed[0] += plain_cols * EXP_DVE_FRAC
                    first_pair = pairs[0][0] if pairs else n_cols
                    # the first groups of each head run all-ScalarE: around
                    # head boundaries the pipeline lead collapses and the
                    # DVE leg's latency would stall PV; the owed columns
                    # shift to mid-head groups where there is slack
                    if g_idx < 2:
                        first_pair = 0
                    c = min(int(dve_owed[0]), first_pair)
                    dve_owed[0] -= c
                    if c > 0:
                        nc.vector.tensor_scalar(
                            p_sb[:, 0:c].bitcast(U16),
                            s_ps[:, 0:c],
                            SCHRAUDOLPH_A,
                            SCHRAUDOLPH_B,
                            mybir.AluOpType.mult,
                            mybir.AluOpType.add,
                        )
                    if c < n_cols:
                        nc.scalar.activation(
                            p_sb[:, c:n_cols],
                            s_ps[:, c:n_cols],
                            mybir.ActivationFunctionType.Exp,
                            scale=scale,
                        )
                    for pos, w in pairs:
                        nc.vector.tensor_mul(
                            p_sb[:, pos : pos + w],
                            p_sb[:, pos : pos + w],
                            tri_sb[:, :w],
                        )
                    if pend is not None:
                        pend[0](pend[1], pend[2])
                    pend = (emit_pv, p_sb, g_items)
                # flush the last group's PV + norm
                pend[0](pend[1], pend[2])
                ctxs.clear()
                loads.clear()

            if repeat > 1:
                with tc.For_i(
                    0,
                    repeat,
                    1,
                    hint_engines=(
                        mybir.EngineType.PE,
                        mybir.EngineType.Activation,
                        mybir.EngineType.DVE,
                        mybir.EngineType.SP,
                        mybir.EngineType.Pool,
                    ),
                ):
                    body()
            else:
                body()

    nc.compile()
    return nc


def _causal_mask_ok(mask: np.ndarray) -> bool:
    m = np.asarray(mask).reshape(S, S)
    expect = np.triu(np.ones((S, S), dtype=bool), k=1)
    return bool((m == expect).all())


def _numpy_fallback(keys, queries, values, mask):
    """Host reference for non-causal masks (robustness insurance)."""
    out = np.empty((B, H, S, D), dtype=np.float32)
    scale = 1.0 / np.sqrt(np.float32(D))
    m = np.asarray(mask).reshape(S, S)
    for b in range(B):
        for h in range(H):
            logits = (queries[b, h] @ keys[b, h].T) * scale
            logits = np.where(m, -np.inf, logits)
            logits -= logits.max(axis=-1, keepdims=True)
            p = np.exp(logits)
            p /= p.sum(axis=-1, keepdims=True)
            out[b, h] = p @ values[b, h]
    return out


def prepare_in_maps(keys, queries, values):
    keys = np.ascontiguousarray(np.asarray(keys, dtype=np.float32))
    queries = np.ascontiguousarray(np.asarray(queries, dtype=np.float32))
    values = np.ascontiguousarray(np.asarray(values, dtype=np.float32))

    # [B,H,...] -> [64, ...] head-pair-major, then 8 heads per core
    q_flat = queries.reshape(B * H, S, D)
    k_flat = keys.reshape(B * H, S, D)
    v_flat = values.reshape(B * H, S, D)

    in_maps = []
    for c in range(N_CORES):
        sl = slice(c * HEADS_PER_CORE, (c + 1) * HEADS_PER_CORE)
        in_maps.append(make_core_inputs(q_flat[sl], k_flat[sl], v_flat[sl]))
    return in_maps


def make_core_inputs(q, k, v):
    """Per-core in_map from [heads, S, D] fp32 arrays."""
    bf = ml_dtypes.bfloat16
    qT = np.ascontiguousarray(q.transpose(0, 2, 1)).astype(bf)
    kT = np.ascontiguousarray(k.transpose(0, 2, 1)).astype(bf)
    vaug = np.zeros((HEADS_PER_CORE, 128, NKB, VAUG_W), dtype=bf)
    # vaug[h, k_local, kb, :128] = V[h, kb*128 + k_local, :]
    vaug[:, :, :, :D] = (
        v.reshape(HEADS_PER_CORE, NKB, 128, D).transpose(0, 2, 1, 3).astype(bf)
    )
    vaug[:, :, :, D] = 1.0
    return {"qT": qT, "kT": kT, "vaug": vaug}


def kernel(keys, queries, values, mask):
    if not _causal_mask_ok(mask):
        return _numpy_fallback(
            np.asarray(keys, dtype=np.float32),
            np.asarray(queries, dtype=np.float32),
            np.asarray(values, dtype=np.float32),
            mask,
        )

    if "nc" not in _COMPILED:
        _COMPILED["nc"] = _build_program()
    nc = _COMPILED["nc"]

    in_maps = prepare_in_maps(keys, queries, values)

    res = None
    last_err = None
    for _attempt in range(3):
        try:
            res = run_bass_kernel_spmd(
                nc, in_maps, core_ids=list(range(N_CORES))
            )
            break
        except Exception as e:  # flaky device state: retry
            last_err = e
    if res is None:
        raise last_err

    out = np.concatenate(
        [res.results[c]["out"][None] for c in range(N_CORES)], axis=0
    )  # [n_cores, heads, S//QCHUNK, 128, OW] permuted, unnormalized
    out = out.reshape(
        N_CORES, HEADS_PER_CORE, S // QCHUNK, 128, NJB, VAUG_W
    )
    o = out[..., :D] / out[..., D : D + 1]  # softmax normalization
    o = o.transpose(0, 1, 2, 4, 3, 5).reshape(B, H, S, D)
    return np.ascontiguousarray(o)

